# revision 1
# baseline (speedup 1.0000x reference)
"""Trainium2 Bass kernel for BertWithAdaThresholdLocContextPooling head.

Data-parallel over batch: 32 batches -> 8 NeuronCores x 4 batches.

v2: byte-minimized + 4-DMA-queue layout for the TRN2 cost model.
  - attention gather rows and the rs-path sequence copy are fp8 (e4m3);
    numerics verified: the context vector rs is a normalized average, so
    fp8 noise is diluted ~sqrt(512)x before it reaches the extractors.
  - extractor weights W_head/W_tail are fp8 with hs-centering: hs ~= c + d
    (c = E[logsumexp of M std normals]); the large common component c is
    routed through an exactly-precomputed f32 row-sum folded into a two-row
    bf16 bias, so fp8 quantization noise only multiplies the small residual
    d. Weights are scaled x16 into the fp8 normal range; the 1/16 is folded
    into the activations (inpT).
  - W_bil stays bf16 (fp8 there costs ~3% rel err).
  - DMAs spread over 4 queues: SP + ACT (HWDGE), DVE (HWDGE, re-enabled),
    Pool (SWDGE: both gathers + a bulk half of seq).

Math per batch b (faithful to the reference, incl. hs in BOTH extractors):
  hs  = logsumexp_m seq[pos[b,0,m]]                       [768]
  A_e = mean_m attention[:, pos[b,e,m], :]                [12, 512]
  w   = sum_h A_0 * A_1;  rs = (w @ seq[b]) / (sum(w) + 12e-5)
  x_f = tanh(W_f @ [hs | rs | ner_f | 1])   f in {head, tail}
  logits = W_bil @ vec(outer-per-group(x_head, x_tail)) + b_bil
"""

import os

import numpy as np

import concourse.bass as bass
import concourse.tile as tile
from concourse import bacc, mybir
from concourse.bass_utils import run_bass_kernel_spmd

# problem dims
B, H, C, D = 32, 12, 512, 768
M = 8
EMB, BLK = 768, 8
NCLS, NER = 97, 6
OFFSET = 1
NCORES = 8
BL = B // NCORES            # batches per core
KP = EMB * BLK              # 6144
NT = KP // 128              # 48 bilinear chunks
NJ = EMB // 128             # 6 emb chunks
CC = 2.578125               # hs centering constant (E[lse of 8 N(0,1)]), bf16-exact
SW = 16.0                   # fp8 weight scale (into e4m3 normal range)
F32 = mybir.dt.float32
BF16 = mybir.dt.bfloat16
F8 = mybir.dt.float8e4
I32 = mybir.dt.int32

# fp8 const block [128, _C8_NCOL]
_C8_SEL96 = 0               # [96,12] mention-mean selector (1/M)
_C8_SEL32 = 12              # [32,4]  mention->batch sum selector
_C8_SAB = 16                # 8 x [64,128] bilinear row replicators (tiled x2)
_C8_NCOL = 16 + 8 * 128
# bf16 const block [128, _CB_NCOL]
_CB_ONESC = 0               # [128,1]
_CB_EYE8 = 1                # [8,4]  [I4; I4] (nb-chunk rhs)
_CB_NEGC = 5                # [128,1] -CC (exp bias column)
_CB_ONE2 = 6                # [2,4] ones (bilinear-bias rhs)
_CB_ONES128 = 10            # [1,128]
_CB_NCOL = 10 + 128
KNB = 8                     # rows of the host-folded ner/bias chunk (hi/lo x 4b)

_CACHE = {}

LAST_EXEC_NS = None
LAST_RESULTS = None

# (quarter, col0, col1, out_base) pieces of each lse d-chunk j over the
# 4-way split mention rows ([128, 192] = 4 quarters x 32 (b,m) x 192 cols)
_LSE_PIECES = [
    [(0, 0, 128, 0)],
    [(0, 128, 192, 0), (1, 0, 64, 64)],
    [(1, 64, 192, 0)],
    [(2, 0, 128, 0)],
    [(2, 128, 192, 0), (3, 0, 64, 64)],
    [(3, 64, 192, 0)],
]


def _build_nc():
    nc = bacc.Bacc("TRN2", target_bir_lowering=False, debug=False)
    # hwdge = {SP, DVE} (the pre-b1a707149 config): the HW supports exactly
    # two HWDGE queues; freeing ACT to run activations un-queued is worth
    # more than its DMA slot (tables+exp+ln+tanh sit on the critical path).
    nc.hwdge_engines.discard(mybir.EngineType.Activation)
    nc.hwdge_engines.add(mybir.EngineType.DVE)
    nc.m.queues = [
        q for q in nc.m.queues if getattr(q, "name", "") != "qActDynamicHW"
    ]
    nc.m.queues.append(
        mybir.DMAQueue(
            type="dynamic",
            name="qDVEDynamicHW",
            blocks=[],
            engine=mybir.EngineType.DVE,
            location_alt=False,
            num_queues=16,
            is_HWDGE=True,
            num_semaphores=0,
            semaphores=[],
        )
    )

    seq8_h = nc.dram_tensor("seq8T", [128, 16 * D], F8, kind="ExternalInput")
    seqbs_h = nc.dram_tensor("seqbs", [BL * C * 4, D // 4], BF16, kind="ExternalInput")
    attn_h = nc.dram_tensor("attn8", [BL * H * C, C], F8, kind="ExternalInput")
    idx_h = nc.dram_tensor("idx", [128, 9], I32, kind="ExternalInput")
    wh8_h = nc.dram_tensor("wh8", [128, 12 * EMB], F8, kind="ExternalInput")
    wt8_h = nc.dram_tensor("wt8", [128, 12 * EMB], F8, kind="ExternalInput")
    # nbT[p, half, j, m]: rows 0..3 = hi(batch p), 4..7 = lo; the host folds
    # W_ner @ ner + bias + CC*rowsum(W_hs) into this per-batch constant.
    # cols [2*NJ*128 : +NCLS] rows 0..1 hold the b_bil (hi, lo) pair.
    nbt_h = nc.dram_tensor("nbT", [KNB, 2 * NJ * 128 + NCLS], BF16, kind="ExternalInput")
    wb_h = nc.dram_tensor("wbT", [128, NT * NCLS], BF16, kind="ExternalInput")
    cst8_h = nc.dram_tensor("cst8", [128, _C8_NCOL], F8, kind="ExternalInput")
    cstb_h = nc.dram_tensor("cstb", [128, _CB_NCOL], BF16, kind="ExternalInput")
    out_h = nc.dram_tensor("outT", [NCLS, BL], F32, kind="ExternalOutput")

    AF = mybir.ActivationFunctionType
    OP = mybir.AluOpType

    with tile.TileContext(nc) as tc:
        with (
            tc.tile_pool(name="w", bufs=1) as wp,
            tc.tile_pool(name="seqp", bufs=1) as sp,
            tc.tile_pool(name="g", bufs=2) as gp,
            tc.tile_pool(name="ps", bufs=8, space="PSUM") as pp,
        ):
            # ---- SP queue: idx (gates gathers), nbT, wh8, wt8-rs, wb piece
            idx_sb = wp.tile([128, 9], I32)
            nc.sync.dma_start(out=idx_sb[:], in_=idx_h[:])
            nbt_sb = wp.tile([KNB, 2 * NJ * 128 + NCLS], BF16)
            nc.sync.dma_start(out=nbt_sb[:], in_=nbt_h[:])
            wh8_sb = wp.tile([128, 12 * EMB], F8)
            nc.sync.dma_start(out=wh8_sb[:], in_=wh8_h[:])
            wt8_sb = wp.tile([128, 12 * EMB], F8)
            nc.sync.dma_start(out=wt8_sb[:, 6 * EMB :], in_=wt8_h[:, 6 * EMB :])
            nc.sync.dma_start(out=wt8_sb[:, : 3 * EMB], in_=wt8_h[:, : 3 * EMB])
            wb_sb = wp.tile([128, NT * NCLS], BF16)
            nc.sync.dma_start(out=wb_sb[:, 32 * NCLS :], in_=wb_h[:, 32 * NCLS :])

            # ---- ACT queue: compute-only. Preload the exp+ln table (set 6)
            # so the table-load pass doesn't first-fit Exp into the tanh set
            ld6 = mybir.InstLoadActFuncSet(act_func_set_id=6)
            ld6.engine = mybir.EngineType.Activation
            nc.scalar.add_instruction(ld6)

            # ---- DVE queue: the whole rs-path sequence in one DMA
            sq = sp.tile([128, 16, D], F8, name="sq")
            nc.vector.dma_start(
                out=sq[:, :, :],
                in_=seq8_h[:, :].rearrange("p (t d) -> p t d", d=D),
            )

            def seqtile(t):
                return sq[:, t, :]

            # ---- Pool queue: consts fill the idx-wait, gathers, wt8-hs, wb
            cstb_sb = wp.tile([128, _CB_NCOL], BF16)
            nc.gpsimd.dma_start(out=cstb_sb[:], in_=cstb_h[:])
            cst8_sb = wp.tile([128, _C8_NCOL], F8)
            nc.gpsimd.dma_start(out=cst8_sb[:], in_=cst8_h[:])

            sel96 = cst8_sb[0 : M * H, _C8_SEL96 : _C8_SEL96 + H]
            onescol = cstb_sb[0:128, _CB_ONESC : _CB_ONESC + 1]
            ones128 = cstb_sb[0:1, _CB_ONES128 : _CB_ONES128 + 128]
            eye8 = cstb_sb[0:KNB, _CB_EYE8 : _CB_EYE8 + BL]
            negc = cstb_sb[0:128, _CB_NEGC : _CB_NEGC + 1]

            # wb pieces fill the remaining idx-wait gap in small steps so
            # the greedy per-queue scheduler cannot slide a long DMA in
            # front of the att gather the moment before idx lands
            nc.gpsimd.dma_start(
                out=wb_sb[:, : 8 * NCLS], in_=wb_h[:, : 8 * NCLS]
            )
            nc.gpsimd.dma_start(
                out=wb_sb[:, 8 * NCLS : 16 * NCLS],
                in_=wb_h[:, 8 * NCLS : 16 * NCLS],
            )
            # gathers (att first: longest downstream chain). One gather per
            # (b,e) slot with a column idx AP: the multi-column batched form
            # returns garbage on real HW (sim-only semantics).
            att_g = gp.tile([M * H, 2 * BL, C], F8, name="att_g")
            g_ment = gp.tile([128, D // 4], BF16, name="g_ment")

            def att_gather(s):
                nc.gpsimd.indirect_dma_start(
                    out=att_g[:, s, :],
                    out_offset=None,
                    in_=attn_h[:],
                    in_offset=bass.IndirectOffsetOnAxis(
                        ap=idx_sb[0 : M * H, s : s + 1], axis=0
                    ),
                )

            # batch-01 slots first, then the mention gather (its exp/ln chain
            # overlaps the batch-23 slots), then the rest
            for s in range(4):
                att_gather(s)
            nc.gpsimd.indirect_dma_start(
                out=g_ment[:, :],
                out_offset=None,
                in_=seqbs_h[:],
                in_offset=bass.IndirectOffsetOnAxis(ap=idx_sb[:, 8:9], axis=0),
            )
            for s in range(4, 8):
                att_gather(s)
            # tail hs-half weight piece b, then the mid wb piece. Both get a
            # fake WAW dependency (a 1-element copy from early gather output,
            # immediately overwritten by the DMA) so the greedy scheduler
            # cannot slide them ahead of the idx-gated gathers.
            nc.vector.tensor_copy(
                wt8_sb[0:1, 3 * EMB : 3 * EMB + 1], att_g[0:1, 0, 0:1]
            )
            nc.gpsimd.dma_start(
                out=wt8_sb[:, 3 * EMB : 6 * EMB], in_=wt8_h[:, 3 * EMB : 6 * EMB]
            )
            nc.vector.tensor_copy(
                wb_sb[0:1, 16 * NCLS : 16 * NCLS + 1], g_ment[0:1, 0:1]
            )
            nc.gpsimd.dma_start(
                out=wb_sb[:, 16 * NCLS : 32 * NCLS],
                in_=wb_h[:, 16 * NCLS : 32 * NCLS],
            )

            scr = wp.tile([1, 2], BF16)

            # ---- phase 1 (ACT part): mention exp, centered by CC via bias
            expm = gp.tile([128, D // 4], BF16, name="expm")
            nc.scalar.activation(expm[:, :], g_ment[:, :], AF.Exp, bias=negc)

            # ---- phases 2+3, split per batch-half: the b01 attention chain
            # (PT -> pte0 -> prod -> wTf -> s -> rs -> inpT) runs while the
            # b23 gather slots are still in flight.
            PT = pp.tile([128, BL, 4, 2, H], F32, tag="ps", name="PT")
            lse_ps = pp.tile([128, NJ, BL], F32, tag="ps", name="lse_ps")
            s_ps = pp.tile([1, BL, 4], F32, tag="ps", name="s_ps")
            sb_ps = pp.tile([128, BL], F32, tag="ps", name="sb_ps")
            rsT = [
                pp.tile([128, 2, BL], F32, tag="ps", name=f"rsT{k}") for k in range(3)
            ]
            inpT = wp.tile([128, 12, BL], BF16)
            lse_ln = wp.tile([128, NJ, BL], F32)
            pte0 = gp.tile([128, BL, 4, H], F32, name="pte0")
            prodT = gp.tile([128, BL, 4, H], F32, name="prodT")
            wT_f = wp.tile([128, BL, 4], BF16)
            sraw = wp.tile([1, BL], F32)
            sden = wp.tile([1, BL], F32)
            srec = wp.tile([1, BL], BF16)
            sb16 = wp.tile([128, 1, BL], BF16)

            for half in range(2):
                sl = slice(2 * half, 2 * half + 2)
                for b in (2 * half, 2 * half + 1):
                    for c in range(4):
                        for e in range(2):
                            nc.tensor.matmul(
                                out=PT[:, b, c, e, :],
                                lhsT=att_g[:, b * 2 + e, c * 128 : (c + 1) * 128],
                                rhs=sel96,
                                start=True,
                                stop=True,
                            )
                nc.vector.tensor_copy(pte0[:, sl, :, :], PT[:, sl, :, 0, :])
                nc.vector.tensor_tensor(
                    out=prodT[:, sl, :, :],
                    in0=pte0[:, sl, :, :],
                    in1=PT[:, sl, :, 1, :],
                    op=OP.mult,
                )
                with nc.allow_low_precision(reason="12-way head sum to bf16"):
                    nc.vector.reduce_sum(
                        out=wT_f[:, sl, :],
                        in_=prodT[:, sl, :, :],
                        axis=mybir.AxisListType.X,
                    )
                nc.tensor.matmul(
                    out=s_ps[:, sl, :],
                    lhsT=onescol,
                    rhs=wT_f[:, sl, :].rearrange("p b c -> p (b c)"),
                    start=True,
                    stop=True,
                )
                nc.vector.reduce_sum(
                    out=sraw[:, sl], in_=s_ps[:, sl, :], axis=mybir.AxisListType.X
                )
                nc.vector.tensor_scalar_add(
                    out=sden[:, sl], in0=sraw[:, sl], scalar1=float(H) * 1e-5
                )
                with nc.allow_low_precision(reason="normalizer rounds to bf16"):
                    nc.vector.reciprocal(out=srec[:, sl], in_=sden[:, sl])
                # rs matmuls emitted before sb_ps so the PE queue is not
                # blocked behind the srec wait
                for b in (2 * half, 2 * half + 1):
                    for j in range(NJ):
                        for c in range(4):
                            nc.tensor.matmul(
                                out=rsT[j % 3][:, j // 3, b : b + 1],
                                lhsT=seqtile(b * 4 + c)[:, j * 128 : (j + 1) * 128],
                                rhs=wT_f[:, b, c : c + 1],
                                start=(c == 0),
                                stop=(c == 3),
                            )
                nc.tensor.matmul(
                    out=sb_ps[:, sl], lhsT=ones128, rhs=srec[:, sl],
                    start=True, stop=True,
                )
                nc.vector.tensor_scalar_mul(
                    out=sb16[:, 0, sl], in0=sb_ps[:, sl], scalar1=1.0 / SW
                )
                for k in range(3):
                    nc.vector.tensor_tensor(
                        out=inpT[:, 6 + k : 12 : 3, sl],
                        in0=rsT[k][:, :, sl],
                        in1=sb16[:, :, sl].to_broadcast([128, 2, 2]),
                        op=OP.mult,
                    )
                if half == 0:
                    # mention logsumexp on the PE/ACT while b23 still gathers
                    for j in range(NJ):
                        for q, c0, c1, ob in _LSE_PIECES[j]:
                            nc.tensor.matmul(
                                out=lse_ps[ob : ob + (c1 - c0), j, :],
                                lhsT=expm[q * 32 : (q + 1) * 32, c0:c1],
                                rhs=cst8_sb[
                                    q * 32 : (q + 1) * 32,
                                    _C8_SEL32 : _C8_SEL32 + BL,
                                ],
                                start=True,
                                stop=True,
                                tile_position=(q * 32, ob),
                            )
                    nc.scalar.activation(lse_ln[:, :, :], lse_ps[:, :, :], AF.Ln)
                    # Tanh-table prefetch pinned after Ln
                    nc.scalar.activation(
                        scr[0:1, 1:2], lse_ln[0:1, 0:1, 0:1], AF.Tanh
                    )
                    nc.vector.tensor_scalar_mul(
                        out=inpT[:, 0:NJ, :], in0=lse_ln[:, :, :], scalar1=1.0 / SW
                    )

            # ---- phase 4: extractors (fp8 weights, bf16 ner/bias chunk)
            # rs chunks first, hs last: the tail hs-half weight (Pool) is the
            # latest-arriving extractor operand
            ex_ps = pp.tile([128, 2, NJ, BL], F32, tag="ps", name="ex_ps")
            corders = (
                list(range(6, 12)) + [12] + list(range(6)),   # head: one sem
                list(range(6)) + [12] + list(range(6, 12)),   # tail: hs first
            )
            for half, w8 in enumerate((wh8_sb, wt8_sb)):
                for j in range(NJ):
                    for ci, c in enumerate(corders[half]):
                        if c < 12:
                            l = w8[:, c * EMB + j * 128 : c * EMB + (j + 1) * 128]
                            r = inpT[:, c, :]
                        else:
                            l = nbt_sb[0:KNB, (half * NJ + j) * 128 : (half * NJ + j + 1) * 128]
                            r = eye8
                        nc.tensor.matmul(
                            out=ex_ps[:, half, j, :],
                            lhsT=l,
                            rhs=r,
                            start=(ci == 0),
                            stop=(ci == 12),
                        )
            ex_fT = wp.tile([128, 2, NJ, BL], BF16)
            nc.scalar.activation(ex_fT[:, :, :, :], ex_ps[:, :, :, :], AF.Tanh)

            # ---- phase 5: grouped bilinear + output matmul
            # one matmul per (side, r): all 6 j-chunks ride as 24 rhs columns
            psA = pp.tile([128, 8, NJ, BL], F32, tag="ps", name="psA")
            psB = pp.tile([128, 8, NJ, BL], F32, tag="ps", name="psB")
            for r in range(8):
                base, v = 64 * (r // 4), r % 4
                selA = cst8_sb[base : base + 64, _C8_SAB + v * 128 : _C8_SAB + (v + 1) * 128]
                selB = cst8_sb[base : base + 64,
                               _C8_SAB + (4 + v) * 128 : _C8_SAB + (5 + v) * 128]
                nc.tensor.matmul(
                    out=psA[:, r, :, :].rearrange("p j b -> p (j b)"), lhsT=selA,
                    rhs=ex_fT[base : base + 64, 0, :, :].rearrange("p j b -> p (j b)"),
                    start=True, stop=True,
                )
                nc.tensor.matmul(
                    out=psB[:, r, :, :].rearrange("p j b -> p (j b)"), lhsT=selB,
                    rhs=ex_fT[base : base + 64, 1, :, :].rearrange("p j b -> p (j b)"),
                    start=True, stop=True,
                )
            # halves pipeline: blT of j 0..2 feeds logits while j 3..5 multiply
            pteA = gp.tile([128, 8, NJ, BL], F32, name="pteA")
            blT = gp.tile([128, 8, NJ, BL], BF16, name="blT")
            logit_ps = pp.tile([NCLS, BL], F32, tag="ps", name="logit_ps")
            # b_bil enters as chunk -1: two bf16 rows (hi/lo) x ones rhs
            nc.tensor.matmul(
                out=logit_ps[:],
                lhsT=nbt_sb[0:2, 2 * NJ * 128 : 2 * NJ * 128 + NCLS],
                rhs=cstb_sb[0:2, _CB_ONE2 : _CB_ONE2 + BL],
                start=True,
                stop=False,
            )
            nc.vector.tensor_copy(pteA[:, :, :, :], psA[:, :, :, :])
            nc.vector.tensor_tensor(
                out=blT[:, :, :, :],
                in0=pteA[:, :, :, :],
                in1=psB[:, :, :, :],
                op=OP.mult,
            )
            for t in range(NT):
                nc.tensor.matmul(
                    out=logit_ps[:],
                    lhsT=wb_sb[:, t * NCLS : (t + 1) * NCLS],
                    rhs=blT[:, t % 8, t // 8, :],
                    start=False,
                    stop=(t == NT - 1),
                )
            logitsT_sb = wp.tile([NCLS, BL], F32)
            nc.vector.tensor_copy(logitsT_sb[:], logit_ps[:])
            nc.sync.dma_start(out=out_h[:], in_=logitsT_sb[:])

    nc.compile()
    return nc


def _bf16(x):
    import ml_dtypes

    return np.ascontiguousarray(np.asarray(x).astype(ml_dtypes.bfloat16))


def _f8(x):
    import ml_dtypes

    return np.ascontiguousarray(np.asarray(x).astype(ml_dtypes.float8_e4m3))


def _weights_prep(W_head, b_head, W_tail, b_tail, W_bil, b_bil):
    """Host-side packing: fp8 x16 transposed main weights, bf16 ner/bias
    chunk with the hs-centering row-sum folded in (two-row bf16 split)."""
    import ml_dtypes

    def bf16_pair(v):
        hi = v.astype(ml_dtypes.bfloat16).astype(np.float32)
        return hi, v - hi

    def pack(Wf, bf):
        Wf = np.asarray(Wf, np.float32)
        w8 = np.zeros((128, 12 * EMB), np.float32)
        wt = Wf.T  # [2D+NER, EMB]
        for c in range(12):
            w8[:, c * EMB : (c + 1) * EMB] = SW * wt[c * 128 : (c + 1) * 128, :]
        # per-extractor constants for the host-folded nb chunk:
        # corr (bias + centering rowsum) and the ner columns
        corr = np.asarray(bf, np.float32) + CC * Wf[:, :D].sum(axis=1)
        return _f8(w8), corr, Wf[:, 2 * D :].astype(np.float32)

    wh8, corr_h, wner_h = pack(W_head, b_head)
    wt8, corr_t, wner_t = pack(W_tail, b_tail)
    bbil_pair = bf16_pair(np.asarray(b_bil, np.float32))

    wbe = np.asarray(W_bil, np.float32).T  # [KP, NCLS]
    wbT = _bf16(wbe.reshape(NT, 128, NCLS).transpose(1, 0, 2).reshape(128, NT * NCLS))

    cst8 = np.zeros((128, _C8_NCOL), np.float32)
    for m in range(M):
        for h in range(H):
            cst8[m * H + h, _C8_SEL96 + h] = 1.0 / M
    for q in range(4):
        for b in range(BL):
            for m in range(M):
                cst8[q * 32 + b * M + m, _C8_SEL32 + b] = 1.0
    p = np.arange(128)
    srcA = (p // 64) * 8 + (p % 64) // 8
    srcB = (p // 64) * 8 + (p % 8)
    sab64 = np.zeros((64, 8 * 128), np.float32)
    for v in range(4):
        sab64[16 * v + srcA, v * 128 + p] = 1.0
        sab64[16 * v + srcB, (4 + v) * 128 + p] = 1.0
    cst8[:, _C8_SAB:] = np.tile(sab64, (2, 1))
    return wh8, wt8, wbT, _f8(cst8), (corr_h, wner_h, corr_t, wner_t, bbil_pair)


def _cstb_prep():
    cstb = np.zeros((128, _CB_NCOL), np.float32)
    cstb[0:128, _CB_ONESC] = 1.0
    for b in range(BL):
        cstb[b, _CB_EYE8 + b] = 1.0
        cstb[BL + b, _CB_EYE8 + b] = 1.0
    cstb[0:128, _CB_NEGC] = -CC
    cstb[0:2, _CB_ONE2 : _CB_ONE2 + BL] = 1.0
    cstb[0:1, _CB_ONES128 : _CB_ONES128 + 128] = 1.0
    return _bf16(cstb)


def _nbt_prep(ner_slice, consts):
    """Per-core [8, 2*NJ*128 + NCLS] bf16 hi/lo pair of the folded ner+bias
    chunk: nb[half][b] = corr_half + W_ner_half @ ner[b, half]."""
    import ml_dtypes

    corr_h, wner_h, corr_t, wner_t, bbil_pair = consts
    nbt = np.zeros((KNB, 2 * NJ * 128 + NCLS), np.float32)
    for half, (corr, wner) in enumerate(((corr_h, wner_h), (corr_t, wner_t))):
        for b in range(BL):
            nb = corr + wner @ ner_slice[b, half]  # [EMB] f32
            hi = nb.astype(ml_dtypes.bfloat16).astype(np.float32)
            nbt[b, half * NJ * 128 : (half + 1) * NJ * 128] = hi
            nbt[BL + b, half * NJ * 128 : (half + 1) * NJ * 128] = nb - hi
    nbt[0, 2 * NJ * 128 :] = bbil_pair[0]
    nbt[1, 2 * NJ * 128 :] = bbil_pair[1]
    return _bf16(nbt)


def _make_in_maps(inputs):
    seq = np.asarray(inputs["sequence_output"], np.float32)
    att = np.asarray(inputs["attention"], np.float32)
    ner = np.asarray(inputs["ner_tags"], np.float32)
    ep = np.asarray(inputs["entity_pos"]).astype(np.int64)
    pos = ep + OFFSET  # [B, 2, M]

    wh8, wt8, wbT, cst8, nbconsts = _weights_prep(
        np.asarray(inputs["W_head"]),
        np.asarray(inputs["b_head"]),
        np.asarray(inputs["W_tail"]),
        np.asarray(inputs["b_tail"]),
        np.asarray(inputs["W_bil"]),
        np.asarray(inputs["b_bil"]),
    )

    in_maps = []
    mh_h = np.tile(np.arange(H), M)    # gather row p = m*H + h -> h
    mh_m = np.repeat(np.arange(M), H)  # -> m
    for k in range(NCORES):
        b0 = k * BL
        seq_k = seq[b0 : b0 + BL].reshape(BL * C, D)
        seq8T = np.zeros((128, 16 * D), np.float32)
        for t in range(16):
            seq8T[:, t * D : (t + 1) * D] = seq_k[t * 128 : (t + 1) * 128, :]
        seqbs = _bf16(seq_k.reshape(BL * C * 4, D // 4))
        att_k = _f8(att[b0 : b0 + BL].reshape(BL * H * C, C))

        idx = np.zeros((128, 9), np.int32)
        for b in range(BL):
            for e in range(2):
                idx[0 : M * H, b * 2 + e] = (b * H + mh_h) * C + pos[b0 + b, e, mh_m]
        for q in range(4):
            for b in range(BL):
                for m in range(M):
                    idx[q * 32 + b * M + m, 8] = (b * C + pos[b0 + b, 0, m]) * 4 + q

        in_maps.append(
            {
                "seq8T": _f8(seq8T),
                "seqbs": seqbs,
                "attn8": att_k,
                "idx": idx,
                "wh8": wh8,
                "nbT": _nbt_prep(ner[b0 : b0 + BL], nbconsts),
                "wt8": wt8,
                "wbT": wbT,
                "cst8": cst8,
                "cstb": _cstb_prep(),
            }
        )
    return in_maps


def _get_nc():
    if "nc" not in _CACHE:
        _CACHE["nc"] = _build_nc()
    return _CACHE["nc"]


def kernel(**inputs):
    global LAST_EXEC_NS, LAST_RESULTS
    nc = _get_nc()
    in_maps = _make_in_maps(inputs)
    trace = bool(int(os.environ.get("BASS_KERNEL_TRACE", "0")))
    try:
        res = run_bass_kernel_spmd(
            nc, in_maps, core_ids=list(range(NCORES)), trace=trace
        )
    except Exception:
        if not trace:
            raise
        # tracing infra unavailable in this environment -- run untraced
        res = run_bass_kernel_spmd(
            nc, in_maps, core_ids=list(range(NCORES)), trace=False
        )
    LAST_EXEC_NS = res.exec_time_ns
    LAST_RESULTS = res
    out = np.zeros((B, NCLS), np.float32)
    for k in range(NCORES):
        out[k * BL : (k + 1) * BL] = np.asarray(res.results[k]["outT"]).T
    return out



# revision 55
# speedup vs baseline: 1.2993x; 1.2993x over previous
"""Trainium2 Bass kernel for BertWithAdaThresholdLocContextPooling head.

Data-parallel over batch: 32 batches -> 8 NeuronCores x 4 batches.

v3: gather consolidation + deadline-ordered DMA placement.
  - attention rows host-packed 3-heads-per-row (1536B rows, heads
    {hp, hp+4, hp+8}): the 8 per-slot 96-row gathers collapse into 2
    128-row gathers (each slot = one aligned 32-partition block), cutting
    the Pool gather stream from ~4us to ~1.2us and landing the last
    attention row ~3us earlier.
  - PT mention-mean matmuls run per (b,c,e,hcol) against a 32-row
    selector replicated across the four 32-partition offsets.
  - PT -> bf16 SBUF copy moved to the ACT engine (both entities in one
    op); the head-product and head-sum run on DVE in bf16 2x mode.
  - sequence DMA split in two (b01 tiles land ~4.3us, b23 ~6.7us);
    weights spread across SP/DVE/Pool by data deadline.
  - extractor accumulation order: hs + nb chunks first, rs chunks last
    (rs activations are the latest-arriving operand).
  - logits DMA'd to DRAM straight from PSUM.

Math per batch b (faithful to the reference, incl. hs in BOTH extractors):
  hs  = logsumexp_m seq[pos[b,0,m]]                       [768]
  A_e = mean_m attention[:, pos[b,e,m], :]                [12, 512]
  w   = sum_h A_0 * A_1;  rs = (w @ seq[b]) / (sum(w) + 12e-5)
  x_f = tanh(W_f @ [hs | rs | ner_f | 1])   f in {head, tail}
  logits = W_bil @ vec(outer-per-group(x_head, x_tail)) + b_bil
"""

import os

import numpy as np

import concourse.bass as bass
import concourse.tile as tile
from concourse import bacc, mybir
from concourse.bass_utils import run_bass_kernel_spmd

# problem dims
B, H, C, D = 32, 12, 512, 768
M = 8
EMB, BLK = 768, 8
NCLS, NER = 97, 6
OFFSET = 1
NCORES = 8
BL = B // NCORES            # batches per core
KP = EMB * BLK              # 6144
NT = KP // 128              # 48 bilinear chunks
NJ = EMB // 128             # 6 emb chunks
CC = 2.578125               # hs centering constant (E[lse of 8 N(0,1)]), bf16-exact
SW = 16.0                   # fp8 weight scale (into e4m3 normal range)
F32 = mybir.dt.float32
BF16 = mybir.dt.bfloat16
F8 = mybir.dt.float8e4
I32 = mybir.dt.int32

# fp8 const block [128, _C8_NCOL]
_C8_SELH = 0                # [128,16] banded (m,hp)->(e,hp) selectors (1/M),
                            # variant v=b%2 at cols 8v (zeros outside the
                            # slot's 32-row band; lhsT stays at base 0)
_C8_SEL32 = 16              # [32,4]x4 mention->batch sum selector (lse)
_C8_WNER = 20               # [12,768] SW*W_ner.T, rows (half,nerdim)
_C8_SAB = 20 + 768          # 8 x [64,128] bilinear row replicators (tiled x2)
_C8_NCOL = 20 + 768 + 8 * 128
# bf16 const block [128, _CB_NCOL]
_CB_ONESC = 0               # [128,1]
_CB_NEGC = 1                # [128,1] -CC (exp bias column)
_CB_ONE2 = 2                # [2,4] ones (bilinear-bias rhs)
_CB_ONES128 = 6             # [1,128]
_CB_CORR = 134              # [24,128] corr hi/lo rows (half,hilo,j)
_CB_CORRSEL = 262           # [24,12*4] per-(half,j) two-row selectors
_CB_NERSEL = 310            # [12,8] ner/SW rhs, variant per half
_CB_BBIL = 318              # [2,97] b_bil (hi,lo)
_CB_NCOL = 318 + 97

_CACHE = {}

LAST_EXEC_NS = None
LAST_RESULTS = None

# (quarter, col0, col1, out_base) pieces of each lse d-chunk j over the
# 4-way split mention rows ([128, 192] = 4 quarters x 32 (b,m) x 192 cols)
_LSE_PIECES = [
    [(0, 0, 128, 0)],
    [(0, 128, 192, 0), (1, 0, 64, 64)],
    [(1, 64, 192, 0)],
    [(2, 0, 128, 0)],
    [(2, 128, 192, 0), (3, 0, 64, 64)],
    [(3, 64, 192, 0)],
]


def _build_nc():
    nc = bacc.Bacc("TRN2", target_bir_lowering=False, debug=False)
    # hwdge = {SP, DVE}: the HW supports exactly two HWDGE queues; ACT is
    # kept un-queued for the activation chain (exp/ln/tanh + PSUM copies).
    nc.hwdge_engines.discard(mybir.EngineType.Activation)
    nc.hwdge_engines.add(mybir.EngineType.DVE)
    nc.m.queues = [
        q for q in nc.m.queues if getattr(q, "name", "") != "qActDynamicHW"
    ]
    nc.m.queues.append(
        mybir.DMAQueue(
            type="dynamic",
            name="qDVEDynamicHW",
            blocks=[],
            engine=mybir.EngineType.DVE,
            location_alt=False,
            num_queues=16,
            is_HWDGE=True,
            num_semaphores=0,
            semaphores=[],
        )
    )

    seq8_h = nc.dram_tensor("seq8T", [128, 16 * D], F8, kind="ExternalInput")
    seqbs_h = nc.dram_tensor("seqbs", [BL * C * 4, D // 4], BF16, kind="ExternalInput")
    attn_h = nc.dram_tensor("attn3", [BL * 4 * C, 3 * C], F8, kind="ExternalInput")
    idx_h = nc.dram_tensor("idx", [128, 3], I32, kind="ExternalInput")
    wh8_h = nc.dram_tensor("wh8", [128, 12 * EMB], F8, kind="ExternalInput")
    wt8_h = nc.dram_tensor("wt8", [128, 12 * EMB], F8, kind="ExternalInput")
    wb_h = nc.dram_tensor("wbT", [128, NT * NCLS], BF16, kind="ExternalInput")
    cst8_h = nc.dram_tensor("cst8", [128, _C8_NCOL], F8, kind="ExternalInput")
    cstb_h = nc.dram_tensor("cstb", [128, _CB_NCOL], BF16, kind="ExternalInput")
    out_h = nc.dram_tensor("outT", [NCLS, BL], F32, kind="ExternalOutput")

    AF = mybir.ActivationFunctionType
    OP = mybir.AluOpType

    with tile.TileContext(nc) as tc:
        with (
            tc.tile_pool(name="w", bufs=1) as wp,
            tc.tile_pool(name="seqp", bufs=1) as sp,
            tc.tile_pool(name="g", bufs=2) as gp,
            tc.tile_pool(name="ps", bufs=8, space="PSUM") as pp,
        ):
            # ---- SP queue: idx (gates gathers) then weights by deadline:
            # both hs-halves, a third of the rs weights, then early wb blocks
            idx_sb = wp.tile([128, 3], I32)
            nc.sync.dma_start(out=idx_sb[:], in_=idx_h[:])
            cst8_sb = wp.tile([128, _C8_NCOL], F8)
            nc.sync.dma_start(
                out=cst8_sb[:, _C8_WNER:], in_=cst8_h[:, _C8_WNER:]
            )
            wh8_sb = wp.tile([128, 12 * EMB], F8)
            nc.sync.dma_start(out=wh8_sb[:, : 6 * EMB], in_=wh8_h[:, : 6 * EMB])
            wt8_sb = wp.tile([128, 12 * EMB], F8)
            nc.sync.dma_start(out=wt8_sb[:, : 6 * EMB], in_=wt8_h[:, : 6 * EMB])
            nc.sync.dma_start(
                out=wh8_sb[:, 6 * EMB : 9 * EMB], in_=wh8_h[:, 6 * EMB : 9 * EMB]
            )
            wb_sb = wp.tile([128, NT * NCLS], BF16)
            nc.sync.dma_start(out=wb_sb[:, : 16 * NCLS], in_=wb_h[:, : 16 * NCLS])
            nc.sync.dma_start(
                out=wb_sb[:, 16 * NCLS : 24 * NCLS],
                in_=wb_h[:, 16 * NCLS : 24 * NCLS],
            )

            # ---- ACT: preload the exp+ln table (set 6) so the table-load
            # pass doesn't first-fit Exp into the tanh set
            ld6 = mybir.InstLoadActFuncSet(act_func_set_id=6)
            ld6.engine = mybir.EngineType.Activation
            nc.scalar.add_instruction(ld6)

            # ---- DVE queue: sequence tiles 0..11 plus two rs-weight thirds,
            # all landing before the DVE compute window opens (~5.5us)
            sq = sp.tile([128, 16, D], F8, name="sq")
            nc.vector.dma_start(
                out=sq[:, 0:8, :],
                in_=seq8_h[:, : 8 * D].rearrange("p (t d) -> p t d", d=D),
            )
            nc.vector.dma_start(
                out=sq[:, 8:12, :],
                in_=seq8_h[:, 8 * D : 12 * D].rearrange("p (t d) -> p t d", d=D),
            )
            nc.vector.dma_start(
                out=wt8_sb[:, 6 * EMB : 9 * EMB], in_=wt8_h[:, 6 * EMB : 9 * EMB]
            )

            def seqtile(t):
                return sq[:, t, :]

            # ---- Pool queue: consts + nbt fill the idx-wait, then gathers
            cstb_sb = wp.tile([128, _CB_NCOL], BF16)
            nc.gpsimd.dma_start(out=cstb_sb[:], in_=cstb_h[:])
            nc.gpsimd.dma_start(out=cst8_sb[:, :_C8_WNER], in_=cst8_h[:, :_C8_WNER])

            onescol = cstb_sb[0:128, _CB_ONESC : _CB_ONESC + 1]
            ones128 = cstb_sb[0:1, _CB_ONES128 : _CB_ONES128 + 128]
            negc = cstb_sb[0:128, _CB_NEGC : _CB_NEGC + 1]

            # gathers: att g0, mention, att g1 (balances the b23 product
            # chain against the exp/ln/table-load chain)
            att_g = gp.tile([128, 2, 3 * C], F8, name="att_g")
            g_ment = gp.tile([128, D // 4], BF16, name="g_ment")

            nc.gpsimd.indirect_dma_start(
                out=att_g[:, 0, :],
                out_offset=None,
                in_=attn_h[:],
                in_offset=bass.IndirectOffsetOnAxis(ap=idx_sb[:, 0:1], axis=0),
            )
            nc.gpsimd.indirect_dma_start(
                out=g_ment[:, :],
                out_offset=None,
                in_=seqbs_h[:],
                in_offset=bass.IndirectOffsetOnAxis(ap=idx_sb[:, 2:3], axis=0),
            )
            nc.gpsimd.indirect_dma_start(
                out=att_g[:, 1, :],
                out_offset=None,
                in_=attn_h[:],
                in_offset=bass.IndirectOffsetOnAxis(ap=idx_sb[:, 1:2], axis=0),
            )
            # late Pool DMAs, each gated on idx via a fake WAW dep (1-element
            # copy, immediately overwritten) so they tie with the gathers in
            # readiness and emission order keeps them behind the gathers.
            nc.gpsimd.tensor_copy(sq[0:1, 12, 0:1], idx_sb[0:1, 0:1])
            nc.gpsimd.dma_start(
                out=sq[:, 12:16, :],
                in_=seq8_h[:, 12 * D :].rearrange("p (t d) -> p t d", d=D),
            )
            nc.gpsimd.tensor_copy(
                wt8_sb[0:1, 9 * EMB : 9 * EMB + 1], idx_sb[0:1, 1:2]
            )
            nc.gpsimd.dma_start(
                out=wt8_sb[:, 9 * EMB :], in_=wt8_h[:, 9 * EMB :]
            )
            nc.gpsimd.tensor_copy(
                wh8_sb[0:1, 9 * EMB : 9 * EMB + 1], idx_sb[0:1, 2:3]
            )
            nc.gpsimd.dma_start(
                out=wh8_sb[:, 9 * EMB :], in_=wh8_h[:, 9 * EMB :]
            )
            nc.gpsimd.tensor_copy(
                wb_sb[0:1, 24 * NCLS : 24 * NCLS + 1], idx_sb[0:1, 2:3]
            )
            nc.gpsimd.dma_start(
                out=wb_sb[:, 24 * NCLS : 40 * NCLS],
                in_=wb_h[:, 24 * NCLS : 40 * NCLS],
            )
            nc.gpsimd.tensor_copy(
                wb_sb[0:1, 40 * NCLS : 40 * NCLS + 1], idx_sb[0:1, 0:1]
            )
            nc.gpsimd.dma_start(
                out=wb_sb[:, 40 * NCLS :], in_=wb_h[:, 40 * NCLS :]
            )

            scr = wp.tile([1, 2], BF16)

            # ---- phase 1 (ACT part): mention exp, centered by CC via bias
            expm = gp.tile([128, D // 4], BF16, name="expm")
            nc.scalar.activation(expm[:, :], g_ment[:, :], AF.Exp, bias=negc)

            # ---- phases 2+3 per batch-half: mention-mean -> head product ->
            # head sum -> normalizer -> rs matmuls -> inpT
            PT = pp.tile([128, BL, 4, 3, 8], F32, tag="ps", name="PT")
            lse_ps = pp.tile([128, NJ, BL], F32, tag="ps", name="lse_ps")
            s_ps = pp.tile([1, BL, 4], F32, tag="ps", name="s_ps")
            sb_ps = pp.tile([128, BL], F32, tag="ps", name="sb_ps")
            rsT = pp.tile([128, NJ, BL], F32, tag="ps", name="rsT")
            inpT = wp.tile([128, 12, BL], BF16)
            lse_ln = wp.tile([128, NJ, BL], F32)
            pteB = wp.tile([128, BL, 4, 3, 8], BF16)
            prodT = gp.tile([128, BL, 4, 3, 4], BF16, name="prodT")
            wT_f = wp.tile([128, BL, 4], BF16)
            sraw = wp.tile([1, BL], F32)
            sden = wp.tile([1, BL], F32)
            srec = wp.tile([1, BL], BF16)
            sb16 = wp.tile([128, 1, BL], BF16)

            # phase A: all PT matmuls + pteB copies first (the PE stream must
            # not strand half-1's PT matmuls behind half-0's dependent
            # matmuls), then the lse/Ln block, then the tanh-table prefetch.
            for half in range(2):
                sl = slice(2 * half, 2 * half + 2)
                for b in (2 * half, 2 * half + 1):
                    v, g = b % 2, b // 2
                    for c in range(4):
                        for k in range(3):
                            nc.tensor.matmul(
                                out=PT[:, b, c, k, :],
                                lhsT=att_g[
                                    :, g, k * C + c * 128 : k * C + (c + 1) * 128
                                ],
                                rhs=cst8_sb[
                                    :, _C8_SELH + 8 * v : _C8_SELH + 8 * v + 8
                                ],
                                start=True,
                                stop=True,
                            )
                with nc.allow_low_precision(reason="mention means to bf16"):
                    nc.scalar.activation(
                        pteB[:, sl, :, :, :], PT[:, sl, :, :, :], AF.Copy
                    )
            # mention logsumexp + Ln
            for j in range(NJ):
                for q, c0, c1, ob in _LSE_PIECES[j]:
                    nc.tensor.matmul(
                        out=lse_ps[ob : ob + (c1 - c0), j, :],
                        lhsT=expm[q * 32 : (q + 1) * 32, c0:c1],
                        rhs=cst8_sb[
                            q * 32 : (q + 1) * 32, _C8_SEL32 : _C8_SEL32 + BL
                        ],
                        start=True,
                        stop=True,
                        tile_position=(q * 32, ob),
                    )
            nc.scalar.activation(lse_ln[:, :, :], lse_ps[:, :, :], AF.Ln)
            nc.vector.tensor_scalar_mul(
                out=inpT[:, 0:NJ, :], in0=lse_ln[:, :, :], scalar1=1.0 / SW
            )
            # Tanh-table prefetch: reads lse_ln (so it follows Ln) and is
            # WAW-pinned after the last pteB copy, so the 1283ns table load
            # fills the ACT gap without blocking Ln or the pteB copies.
            nc.vector.tensor_copy(scr[0:1, 1:2], pteB[0:1, 2, 0:1, 0, 0:1])
            nc.scalar.activation(scr[0:1, 1:2], lse_ln[0:1, 0:1, 0:1], AF.Tanh)

            # phase B: per half, head product -> head sum -> normalizer ->
            # rs matmuls -> inpT
            for half in range(2):
                sl = slice(2 * half, 2 * half + 2)
                nc.vector.tensor_tensor(
                    out=prodT[:, sl, :, :, :],
                    in0=pteB[:, sl, :, :, 0:4],
                    in1=pteB[:, sl, :, :, 4:8],
                    op=OP.mult,
                )
                with nc.allow_low_precision(reason="12-way head sum to bf16"):
                    nc.vector.reduce_sum(
                        out=wT_f[:, sl, :],
                        in_=prodT[:, sl, :, :, :].rearrange(
                            "p b c k h -> p b c (k h)"
                        ),
                        axis=mybir.AxisListType.X,
                    )
                nc.tensor.matmul(
                    out=s_ps[:, sl, :],
                    lhsT=onescol,
                    rhs=wT_f[:, sl, :].rearrange("p b c -> p (b c)"),
                    start=True,
                    stop=True,
                )
                nc.vector.reduce_sum(
                    out=sraw[:, sl], in_=s_ps[:, sl, :], axis=mybir.AxisListType.X
                )
                # the reference's +1e-5 is ~1e-7 relative to s (sum of ~512
                # positive head-products); dropping it is far below the
                # fp8/bf16 noise floor and removes a hop from the chain
                with nc.allow_low_precision(reason="normalizer rounds to bf16"):
                    nc.vector.reciprocal(out=srec[:, sl], in_=sraw[:, sl])
                # rs matmuls emitted before sb_ps so the PE queue is not
                # blocked behind the srec wait
                for b in (2 * half, 2 * half + 1):
                    for j in range(NJ):
                        for c in range(4):
                            nc.tensor.matmul(
                                out=rsT[:, j, b : b + 1],
                                lhsT=seqtile(b * 4 + c)[:, j * 128 : (j + 1) * 128],
                                rhs=wT_f[:, b, c : c + 1],
                                start=(c == 0),
                                stop=(c == 3),
                            )
                nc.tensor.matmul(
                    out=sb_ps[:, sl], lhsT=ones128, rhs=srec[:, sl],
                    start=True, stop=True,
                )
                nc.vector.tensor_scalar_mul(
                    out=sb16[:, 0, sl], in0=sb_ps[:, sl], scalar1=1.0 / SW
                )
                nc.vector.tensor_tensor(
                    out=inpT[:, 6:12, sl],
                    in0=rsT[:, :, sl],
                    in1=sb16[:, :, sl].to_broadcast([128, NJ, 2]),
                    op=OP.mult,
                )

            # ---- phase 4: extractors (fp8 weights, bf16 ner/bias chunk)
            # hs + nb chunks first, rs chunks last (latest-arriving operand)
            ex_ps = pp.tile([128, 2, NJ, BL], F32, tag="ps", name="ex_ps")
            corder = list(range(6)) + [12, 13] + list(range(6, 12))
            for half, w8 in enumerate((wh8_sb, wt8_sb)):
                for j in range(NJ):
                    for ci, c in enumerate(corder):
                        if c < 12:
                            l = w8[:, c * EMB + j * 128 : c * EMB + (j + 1) * 128]
                            r = inpT[:, c, :]
                        elif c == 12:
                            # corr (bias + centering + hi/lo) via 24-row
                            # banded selector
                            l = cstb_sb[0:24, _CB_CORR : _CB_CORR + 128]
                            r = cstb_sb[
                                0:24,
                                _CB_CORRSEL
                                + (half * NJ + j) * BL : _CB_CORRSEL
                                + (half * NJ + j + 1) * BL,
                            ]
                        else:
                            # W_ner @ ner via 12-row banded ner rhs
                            l = cst8_sb[0:12, _C8_WNER + j * 128 : _C8_WNER + (j + 1) * 128]
                            r = cstb_sb[
                                0:12,
                                _CB_NERSEL + half * BL : _CB_NERSEL + (half + 1) * BL,
                            ]
                        nc.tensor.matmul(
                            out=ex_ps[:, half, j, :],
                            lhsT=l,
                            rhs=r,
                            start=(ci == 0),
                            stop=(ci == 13),
                        )
            ex_fT = wp.tile([128, 2, NJ, BL], BF16)
            nc.scalar.activation(ex_fT[:, :, :, :], ex_ps[:, :, :, :], AF.Tanh)

            # ---- phase 5: grouped bilinear + output matmul, split by
            # replicator halves so ACT copy / DVE product / PE logit chunks
            # pipeline.
            psA = pp.tile([128, 8, NJ, BL], F32, tag="ps", name="psA")
            psB = pp.tile([128, 8, NJ, BL], F32, tag="ps", name="psB")

            def repl_mm(side, r):
                base, v = 64 * (r // 4), r % 4
                sel = cst8_sb[
                    base : base + 64,
                    _C8_SAB + (4 * side + v) * 128 : _C8_SAB + (4 * side + v + 1) * 128,
                ]
                ps = psA if side == 0 else psB
                nc.tensor.matmul(
                    out=ps[:, r, :, :].rearrange("p j b -> p (j b)"), lhsT=sel,
                    rhs=ex_fT[base : base + 64, side, :, :].rearrange(
                        "p j b -> p (j b)"
                    ),
                    start=True, stop=True,
                )

            for r in range(4):
                repl_mm(0, r)
            for r in range(4):
                repl_mm(1, r)
            for r in range(4, 8):
                repl_mm(0, r)
            for r in range(4, 8):
                repl_mm(1, r)
            pteA = gp.tile([128, 8, NJ, BL], BF16, name="pteA")
            blT = gp.tile([128, 8, NJ, BL], BF16, name="blT")
            logit_ps = pp.tile([NCLS, BL], F32, tag="ps", name="logit_ps")
            # b_bil enters as chunk -1: two bf16 rows (hi/lo) x ones rhs
            nc.tensor.matmul(
                out=logit_ps[:],
                lhsT=cstb_sb[0:2, _CB_BBIL : _CB_BBIL + NCLS],
                rhs=cstb_sb[0:2, _CB_ONE2 : _CB_ONE2 + BL],
                start=True,
                stop=False,
            )
            for rh in range(2):
                rs_ = slice(4 * rh, 4 * rh + 4)
                with nc.allow_low_precision(reason="bilinear factors to bf16"):
                    nc.scalar.activation(
                        pteA[:, rs_, :, :], psA[:, rs_, :, :], AF.Copy
                    )
                nc.vector.tensor_tensor(
                    out=blT[:, rs_, :, :],
                    in0=pteA[:, rs_, :, :],
                    in1=psB[:, rs_, :, :],
                    op=OP.mult,
                )
                # wb blocks are host-laid in this consumption order, so the
                # last-arriving wb DMA pieces feed the last matmuls
                for j in range(NJ):
                    for r in range(4 * rh, 4 * rh + 4):
                        i = rh * 24 + j * 4 + (r - 4 * rh)
                        nc.tensor.matmul(
                            out=logit_ps[:],
                            lhsT=wb_sb[:, i * NCLS : (i + 1) * NCLS],
                            rhs=blT[:, r, j, :],
                            start=False,
                            stop=(rh == 1 and j == NJ - 1 and r == 7),
                        )
            logitsT_sb = wp.tile([NCLS, BL], F32)
            nc.vector.tensor_copy(logitsT_sb[:], logit_ps[:])
            nc.sync.dma_start(out=out_h[:], in_=logitsT_sb[:])

    nc.compile()
    return nc


def _bf16(x):
    import ml_dtypes

    return np.ascontiguousarray(np.asarray(x).astype(ml_dtypes.bfloat16))


def _f8(x):
    import ml_dtypes

    return np.ascontiguousarray(np.asarray(x).astype(ml_dtypes.float8_e4m3))


def _weights_prep(W_head, b_head, W_tail, b_tail, W_bil, b_bil):
    """Host-side packing: fp8 x16 transposed main weights, bf16 ner/bias
    chunk with the hs-centering row-sum folded in (two-row bf16 split)."""
    import ml_dtypes

    def bf16_pair(v):
        hi = v.astype(ml_dtypes.bfloat16).astype(np.float32)
        return hi, v - hi

    def pack(Wf, bf):
        Wf = np.asarray(Wf, np.float32)
        w8 = np.zeros((128, 12 * EMB), np.float32)
        wt = Wf.T  # [2D+NER, EMB]
        for c in range(12):
            w8[:, c * EMB : (c + 1) * EMB] = SW * wt[c * 128 : (c + 1) * 128, :]
        # per-extractor constants for the host-folded nb chunk:
        # corr (bias + centering rowsum) and the ner columns
        corr = np.asarray(bf, np.float32) + CC * Wf[:, :D].sum(axis=1)
        return _f8(w8), corr, Wf[:, 2 * D :].astype(np.float32)

    wh8, corr_h, wner_h = pack(W_head, b_head)
    wt8, corr_t, wner_t = pack(W_tail, b_tail)
    bbil_pair = bf16_pair(np.asarray(b_bil, np.float32))

    wbe = np.asarray(W_bil, np.float32).T  # [KP, NCLS]
    wbc = wbe.reshape(NT, 128, NCLS).transpose(1, 0, 2)  # [128, t, NCLS]
    # consumption order: block i = (rh, j, r) -> canonical chunk t = j*8+r
    perm = [
        (j * 8 + 4 * rh + rr)
        for rh in range(2)
        for j in range(NJ)
        for rr in range(4)
    ]
    wbT = _bf16(wbc[:, perm, :].reshape(128, NT * NCLS))

    cst8 = np.zeros((128, _C8_NCOL), np.float32)
    for v in range(2):
        for e in range(2):
            for m in range(M):
                for hp in range(4):
                    cst8[
                        64 * v + 32 * e + m * 4 + hp, _C8_SELH + 8 * v + 4 * e + hp
                    ] = 1.0 / M
    for q in range(4):
        for b in range(BL):
            for m in range(M):
                cst8[q * 32 + b * M + m, _C8_SEL32 + b] = 1.0
    p = np.arange(128)
    srcA = (p // 64) * 8 + (p % 64) // 8
    srcB = (p // 64) * 8 + (p % 8)
    sab64 = np.zeros((64, 8 * 128), np.float32)
    for v in range(4):
        sab64[16 * v + srcA, v * 128 + p] = 1.0
        sab64[16 * v + srcB, (4 + v) * 128 + p] = 1.0
    cst8[:, _C8_SAB:] = np.tile(sab64, (2, 1))
    return wh8, wt8, wbT, _f8(cst8), (corr_h, wner_h, corr_t, wner_t, bbil_pair)


def _cstb_prep():
    cstb = np.zeros((128, _CB_NCOL), np.float32)
    cstb[0:128, _CB_ONESC] = 1.0
    for b in range(BL):
        cstb[b, _CB_EYE8 + b] = 1.0
        cstb[BL + b, _CB_EYE8 + b] = 1.0
    cstb[0:128, _CB_NEGC] = -CC
    cstb[0:2, _CB_ONE2 : _CB_ONE2 + BL] = 1.0
    cstb[0:1, _CB_ONES128 : _CB_ONES128 + 128] = 1.0
    return _bf16(cstb)


def _nbt_prep(ner_slice, consts):
    """Per-core [8, 2*NJ*128 + NCLS] bf16 hi/lo pair of the folded ner+bias
    chunk: nb[half][b] = corr_half + W_ner_half @ ner[b, half]."""
    import ml_dtypes

    corr_h, wner_h, corr_t, wner_t, bbil_pair = consts
    nbt = np.zeros((KNB, 2 * NJ * 128 + NCLS), np.float32)
    for half, (corr, wner) in enumerate(((corr_h, wner_h), (corr_t, wner_t))):
        for b in range(BL):
            nb = corr + wner @ ner_slice[b, half]  # [EMB] f32
            hi = nb.astype(ml_dtypes.bfloat16).astype(np.float32)
            nbt[b, half * NJ * 128 : (half + 1) * NJ * 128] = hi
            nbt[BL + b, half * NJ * 128 : (half + 1) * NJ * 128] = nb - hi
    nbt[0, 2 * NJ * 128 :] = bbil_pair[0]
    nbt[1, 2 * NJ * 128 :] = bbil_pair[1]
    return _bf16(nbt)


def _make_in_maps(inputs):
    seq = np.asarray(inputs["sequence_output"], np.float32)
    att = np.asarray(inputs["attention"], np.float32)
    ner = np.asarray(inputs["ner_tags"], np.float32)
    ep = np.asarray(inputs["entity_pos"]).astype(np.int64)
    pos = ep + OFFSET  # [B, 2, M]

    wh8, wt8, wbT, cst8, nbconsts = _weights_prep(
        np.asarray(inputs["W_head"]),
        np.asarray(inputs["b_head"]),
        np.asarray(inputs["W_tail"]),
        np.asarray(inputs["b_tail"]),
        np.asarray(inputs["W_bil"]),
        np.asarray(inputs["b_bil"]),
    )

    in_maps = []
    for k in range(NCORES):
        b0 = k * BL
        seq_k = seq[b0 : b0 + BL].reshape(BL * C, D)
        seq8T = np.zeros((128, 16 * D), np.float32)
        for t in range(16):
            seq8T[:, t * D : (t + 1) * D] = seq_k[t * 128 : (t + 1) * 128, :]
        seqbs = _bf16(seq_k.reshape(BL * C * 4, D // 4))
        # 3-head packed attention rows: row (b, hp, c) = att[b, {hp, hp+4,
        # hp+8}, c, :] concatenated (1536B rows)
        att_k = att[b0 : b0 + BL]  # [BL, H, C, C]
        att3 = np.empty((BL, 4, C, 3 * C), np.float32)
        for hp in range(4):
            for kk in range(3):
                att3[:, hp, :, kk * C : (kk + 1) * C] = att_k[:, hp + 4 * kk]
        att3 = _f8(att3.reshape(BL * 4 * C, 3 * C))

        idx = np.zeros((128, 3), np.int32)
        for b in range(BL):
            for e in range(2):
                s = b * 2 + e
                q, g = s % 4, s // 4
                for m in range(M):
                    for hp in range(4):
                        idx[32 * q + m * 4 + hp, g] = (b * 4 + hp) * C + pos[
                            b0 + b, e, m
                        ]
        for q in range(4):
            for b in range(BL):
                for m in range(M):
                    idx[q * 32 + b * M + m, 2] = (b * C + pos[b0 + b, 0, m]) * 4 + q

        in_maps.append(
            {
                "seq8T": _f8(seq8T),
                "seqbs": seqbs,
                "attn3": att3,
                "idx": idx,
                "wh8": wh8,
                "nbT": _nbt_prep(ner[b0 : b0 + BL], nbconsts),
                "wt8": wt8,
                "wbT": wbT,
                "cst8": cst8,
                "cstb": _cstb_prep(),
            }
        )
    return in_maps


def _get_nc():
    if "nc" not in _CACHE:
        _CACHE["nc"] = _build_nc()
    return _CACHE["nc"]


def kernel(**inputs):
    global LAST_EXEC_NS, LAST_RESULTS
    nc = _get_nc()
    in_maps = _make_in_maps(inputs)
    trace = bool(int(os.environ.get("BASS_KERNEL_TRACE", "0")))
    try:
        res = run_bass_kernel_spmd(
            nc, in_maps, core_ids=list(range(NCORES)), trace=trace
        )
    except Exception:
        if not trace:
            raise
        # tracing infra unavailable in this environment -- run untraced
        res = run_bass_kernel_spmd(
            nc, in_maps, core_ids=list(range(NCORES)), trace=False
        )
    LAST_EXEC_NS = res.exec_time_ns
    LAST_RESULTS = res
    out = np.zeros((B, NCLS), np.float32)
    for k in range(NCORES):
        out[k * BL : (k + 1) * BL] = np.asarray(res.results[k]["outT"]).T
    return out
